# revision 14
# baseline (speedup 1.0000x reference)
"""Trainium2 Bass kernel for the 4-gate top-2 MoE block (dense all-expert).

Strategy: pure data parallelism — sample b -> NeuronCore b (B == 8 == n_cores).
Each core runs the full gating + all-8-expert 1x1-conv stack + gated combine
for its own sample; outputs are concatenated on the host.  The load-balance
loss needs the softmax probs of all samples, so each core DMAs out its tiny
(4,8) probs tile and the final 8-value variance reduction happens on the host.

Matmuls run in float32r mode (4x the fp32 PE throughput).  The PE rounds
f32r operands internally; producers only have to be f32r-typed, so weights
and activations stream over the two hardware DGE rings as bitcast views —
no software (casting) DMA anywhere on the critical path.

DMA ring assignment:
  sync  (qSP HWDGE):  gates/biases, per-expert weights, per-chunk x (f32r)
  scalar(qAct HWDGE): per-chunk x (f32 copy for the exact global-avg-pool),
                      per-chunk output stores

Set MM_DTYPE = "f32" below for a full-precision fp32 fallback.
"""

import numpy as np

import concourse.bacc as bacc
import concourse.mybir as mybir
import concourse.tile as tile
from concourse.bass_utils import run_bass_kernel_spmd

F32 = mybir.dt.float32
F32R = mybir.dt.float32r
AF = mybir.ActivationFunctionType
ALU = mybir.AluOpType
AX = mybir.AxisListType

# problem shape (hardcoded; spec nn_MoE_77120432767217)
B, C, H, W = 8, 256, 56, 56
HW = H * W              # 3136
E, HID, G = 8, 512, 4
BN_EPS = 1e-5
CT = C // 128           # 2   c-tiles
HT = HID // 128         # 4   hid-tiles
NSZ = 392               # free-dim chunk
NCH = HW // NSZ         # 8   chunks
NCORES = 8

MM_DTYPE = "f32r"       # "f32r" (fast, tf32-ish) or "f32" (full precision)

_CACHE = {}


def _build_nc():
    """Build + compile the per-core Bass program (SPMD: same program, 8 cores)."""
    mmdt = F32R if MM_DTYPE == "f32r" else F32

    def asmm(ap):   # f32 dram view -> matmul-dtype view (free relabel)
        return ap.bitcast(mmdt) if mmdt != F32 else ap

    nc = bacc.Bacc("TRN2", target_bir_lowering=False, debug=False,
                   num_devices=NCORES)

    # ---- DRAM parameters (per core) ----
    xd = nc.dram_tensor("xb", [128, NCH, CT, NSZ], F32, kind="ExternalInput").ap()
    w1d = nc.dram_tensor("w1", [128, E, CT, HT, 128], F32, kind="ExternalInput").ap()
    w2d = nc.dram_tensor("w2", [128, E, HT, HT, 128], F32, kind="ExternalInput").ap()
    w3d = nc.dram_tensor("w3", [128, E, HT, CT, 128], F32, kind="ExternalInput").ap()
    b1d = nc.dram_tensor("b1s", [128, E * HT], F32, kind="ExternalInput").ap()
    bnsd = nc.dram_tensor("bns", [128, E * HT], F32, kind="ExternalInput").ap()
    bnbd = nc.dram_tensor("bnb", [128, E * HT], F32, kind="ExternalInput").ap()
    b3d = nc.dram_tensor("b3s", [128, E * CT], F32, kind="ExternalInput").ap()
    gtd = nc.dram_tensor("gt", [128, CT, G * E], F32, kind="ExternalInput").ap()
    outd = nc.dram_tensor("out", [128, NCH, G, CT, NSZ], F32,
                          kind="ExternalOutput").ap()
    probsd = nc.dram_tensor("probs", [1, G * E], F32, kind="ExternalOutput").ap()

    with tile.TileContext(nc) as tc:
        with (
            tc.tile_pool(name="wconst", bufs=1) as wpool,
            tc.tile_pool(name="bconst", bufs=1) as bpool,
            tc.tile_pool(name="xr", bufs=3) as xrpool,
            tc.tile_pool(name="xf", bufs=1) as xfpool,
            tc.tile_pool(name="gat", bufs=1) as gpool,
            tc.tile_pool(name="h1p", bufs=2) as h1pool,
            tc.tile_pool(name="h2p", bufs=2) as h2pool,
            tc.tile_pool(name="eop", bufs=2) as eopool,
            tc.tile_pool(name="outp", bufs=2) as outpool,
            tc.tile_pool(name="ps", bufs=7, space="PSUM") as pspool,
            tc.tile_pool(name="psg", bufs=1, space="PSUM") as psgpool,
        ):
            b1sb = bpool.tile([128, E * HT], F32, tag="b1sb")
            bnssb = bpool.tile([128, E * HT], F32, tag="bnssb")
            bnbsb = bpool.tile([128, E * HT], F32, tag="bnbsb")
            b3sb = bpool.tile([128, E * CT], F32, tag="b3sb")
            gtsb = bpool.tile([128, CT, G * E], F32, tag="gtsb")

            # ---- per-expert weights + rounded x chunks on the sync ring ----
            w1sb = wpool.tile([128, E, CT, HT, 128], mmdt, tag="w1sb")
            w2sb = wpool.tile([128, E, HT, HT, 128], mmdt, tag="w2sb")
            w3sb = wpool.tile([128, E, HT, CT, 128], mmdt, tag="w3sb")

            xrs = []

            # ---- x global-average-pool (exact fp32): two half-x DMAs, one
            # per HWDGE ring, reduced over (chunk, n) via transposed views --
            QJ = NCH // 4
            rsums = []

            def _xf_gap_stream():
                engs = [nc.sync, nc.scalar, nc.sync, nc.scalar]
                for h in range(4):
                    xf = xfpool.tile([128, QJ, CT, NSZ], F32, bufs=2)
                    engs[h].dma_start(
                        out=xf[:], in_=xd[:, h * QJ:(h + 1) * QJ])
                    rs = gpool.tile([128, CT], F32, tag=f"rs{h}")
                    nc.vector.tensor_reduce(out=rs[:],
                                            in_=xf[:].transpose([0, 2, 1, 3]),
                                            axis=AX.XY, op=ALU.add)
                    rsums.append(rs)

            def emit_xr(j):
                xr = xrpool.tile([128, CT, NSZ], mmdt, tag="xr")
                nc.sync.dma_start(out=xr[:], in_=asmm(xd[:, j]))
                xrs.append(xr)

            # warm the ACT table set containing Exp/Relu/Identity while
            # the first DMAs are in flight (saves ~2.7us on the gating chain)
            warm = gpool.tile([1, 1], F32, tag="warm")
            nc.vector.memset(warm[:], 0.0)
            nc.scalar.activation(warm[:], warm[:], AF.Exp)

            # ring heads: expert-0 weights + x chunk 0, then the GAP
            # quarters, gating constants, remaining weights.
            emit_xr(0)
            nc.sync.dma_start(out=w1sb[:, 0], in_=asmm(w1d[:, 0]))
            nc.sync.dma_start(out=w3sb[:, 0], in_=asmm(w3d[:, 0]))
            _xf_gap_stream()
            nc.sync.dma_start(out=gtsb[:], in_=gtd)
            # w2 rides the scalar ring in two bulk transfers so the ACT
            # engine only spends two trigger slots on it
            nc.scalar.dma_start(out=w2sb[:, 0:4], in_=asmm(w2d[:, 0:4]))
            nc.scalar.dma_start(out=w2sb[:, 4:8], in_=asmm(w2d[:, 4:8]))
            for t, d in ((b1sb, b1d), (bnssb, bnsd), (bnbsb, bnbd),
                         (b3sb, b3d)):
                nc.sync.dma_start(out=t[:], in_=d)
            for e in range(1, E):
                nc.sync.dma_start(out=w1sb[:, e], in_=asmm(w1d[:, e]))
                nc.sync.dma_start(out=w3sb[:, e], in_=asmm(w3d[:, e]))
                if e == 1:
                    emit_xr(1)
                if e == 4:
                    emit_xr(2)

            # ---- gating: logits = (sum_hw x / HW) @ gates ----
            xacc = gpool.tile([128, CT], F32, tag="xacc")
            nc.vector.tensor_add(out=xacc[:], in0=rsums[0][:], in1=rsums[1][:])
            for h in range(2, 4):
                nc.vector.tensor_add(out=xacc[:], in0=xacc[:], in1=rsums[h][:])

            lps = psgpool.tile([1, G * E], F32, tag="gps")
            for kt in range(CT):
                nc.tensor.matmul(lps[:], lhsT=xacc[:, kt:kt + 1],
                                 rhs=gtsb[:, kt, :],
                                 start=(kt == 0), stop=(kt == CT - 1))
            lg = gpool.tile([1, G * E], F32, tag="lg")
            nc.vector.tensor_copy(out=lg[:], in_=lps[:])

            def v48(t):   # (1, 32) tile -> (1, 4, 8) view
                return t[:].rearrange("p (g e) -> p g e", e=E)

            def bc(t):    # (1, 4) tile -> (1, 4, 8) broadcast view
                return t[:].unsqueeze(2).broadcast_to((1, G, E))

            # numerically-stable softmax over e (per gate)
            mx = gpool.tile([1, G], F32, tag="mx")
            nc.vector.tensor_reduce(out=mx[:], in_=v48(lg), axis=AX.X, op=ALU.max)
            sh = gpool.tile([1, G * E], F32, tag="sh")
            nc.vector.tensor_tensor(out=v48(sh), in0=v48(lg), in1=bc(mx),
                                    op=ALU.subtract)
            ex = gpool.tile([1, G * E], F32, tag="ex")
            nc.scalar.activation(ex[:], sh[:], AF.Exp)
            sm = gpool.tile([1, G], F32, tag="sm")
            nc.vector.tensor_reduce(out=sm[:], in_=v48(ex), axis=AX.X, op=ALU.add)
            sinv = gpool.tile([1, G], F32, tag="sinv")
            nc.vector.reciprocal(out=sinv[:], in_=sm[:])
            probs = gpool.tile([1, G * E], F32, tag="probs")
            nc.vector.tensor_tensor(out=v48(probs), in0=v48(ex), in1=bc(sinv),
                                    op=ALU.mult)
            nc.sync.dma_start(out=probsd, in_=probs[:])

            # top-2 per gate, selected on LOGITS (same order as probs,
            # but logit gaps are much wider than prob gaps -> robust)
            m1 = gpool.tile([1, G], F32, tag="m1")
            nc.vector.tensor_reduce(out=m1[:], in_=v48(lg), axis=AX.X, op=ALU.max)
            msk = gpool.tile([1, G * E], F32, tag="msk")
            nc.vector.tensor_tensor(out=v48(msk), in0=v48(lg), in1=bc(m1),
                                    op=ALU.is_equal)
            pm = gpool.tile([1, G * E], F32, tag="pm")
            nc.vector.scalar_tensor_tensor(out=v48(pm), in0=v48(msk),
                                           scalar=-1e30, in1=v48(lg),
                                           op0=ALU.mult, op1=ALU.add)
            m2 = gpool.tile([1, G], F32, tag="m2")
            nc.vector.tensor_reduce(out=m2[:], in_=v48(pm), axis=AX.X, op=ALU.max)
            sel = gpool.tile([1, G * E], F32, tag="sel")
            nc.vector.tensor_tensor(out=v48(sel), in0=v48(lg), in1=bc(m2),
                                    op=ALU.is_ge)
            num = gpool.tile([1, G * E], F32, tag="num")
            nc.vector.tensor_tensor(out=num[:], in0=probs[:], in1=sel[:],
                                    op=ALU.mult)
            den = gpool.tile([1, G], F32, tag="den")
            nc.vector.tensor_reduce(out=den[:], in_=v48(num), axis=AX.X, op=ALU.add)
            nc.vector.tensor_scalar_add(out=den[:], in0=den[:], scalar1=1e-10)
            dinv = gpool.tile([1, G], F32, tag="dinv")
            nc.vector.reciprocal(out=dinv[:], in_=den[:])
            cw = gpool.tile([1, G * E], F32, tag="cw")
            nc.vector.tensor_tensor(out=v48(cw), in0=v48(num), in1=bc(dinv),
                                    op=ALU.mult)

            ones = gpool.tile([1, 128], F32, tag="ones")
            nc.vector.memset(ones[:], 1.0)
            cwb = gpool.tile([128, G * E], F32, tag="cwb")

            def emit_cw_broadcast():
                # broadcast cw to all 128 partitions with a K=1 ones-matmul;
                # emitted a little way INTO the expert stream so the PE does
                # not sit on the softmax/top-k chain's latency.
                cwps = psgpool.tile([128, G * E], F32, tag="gps")
                nc.tensor.matmul(cwps[:], lhsT=ones[:], rhs=cw[:],
                                 start=True, stop=True)
                nc.vector.tensor_copy(out=cwb[:], in_=cwps[:])

            # scheduler fence: the gating phase schedules before the expert
            # loop on every engine, so the gated-combine's cw dependency can
            # never wedge the PSUM/PE pipeline below.
            tc.no_sync_barrier()

            # ---- main dense-expert stack, chunk-outer / expert-inner ----
            def emit_accums(eo, e, outg):
                # outg[:, g] (+)= cw[g,e] * eo (deferred by one expert)
                for g in range(G):
                    sc = cwb[:, g * E + e: g * E + e + 1]
                    eng = nc.vector
                    if e == 0:
                        eng.tensor_scalar_mul(out=outg[:, g], in0=eo[:],
                                              scalar1=sc)
                    else:
                        eng.scalar_tensor_tensor(out=outg[:, g], in0=eo[:],
                                                 scalar=sc, in1=outg[:, g],
                                                 op0=ALU.mult, op1=ALU.add)

            pending = None   # previous expert's eo, combined one expert late
            for j in range(NCH):
                if j + 3 < NCH:
                    emit_xr(j + 3)
                outg = outpool.tile([128, G, CT, NSZ], F32)
                for e in range(E):
                    h1 = h1pool.tile([128, HT, NSZ], mmdt)
                    for mt in range(HT):
                        ps = pspool.tile([128, NSZ], F32)
                        for kt in range(CT):
                            nc.tensor.matmul(ps[:],
                                             lhsT=w1sb[:, e, kt, mt, :],
                                             rhs=xrs[j][:, kt, :],
                                             start=(kt == 0),
                                             stop=(kt == CT - 1))
                        bia = b1sb[:, e * HT + mt: e * HT + mt + 1]
                        # e==0: keep DVE free for the previous chunk's
                        # deferred combine; otherwise alternate ACT/DVE
                        if e == 0 or mt % 2 == 0:
                            nc.scalar.add(h1[:, mt, :], ps[:], add=bia)
                        else:
                            nc.vector.tensor_scalar_add(out=h1[:, mt, :],
                                                        in0=ps[:], scalar1=bia)
                    if j == 0 and e == 1:
                        emit_cw_broadcast()
                    if pending is not None:
                        peo, pe_, poutg, pj = pending
                        emit_accums(peo, pe_, poutg)
                        if pe_ == E - 1:
                            nc.sync.dma_start(out=outd[:, pj], in_=poutg[:])
                        pending = None
                    h2 = h2pool.tile([128, HT, NSZ], mmdt)
                    for mt in range(HT):
                        ps = pspool.tile([128, NSZ], F32)
                        for kt in range(HT):
                            nc.tensor.matmul(ps[:],
                                             lhsT=w2sb[:, e, kt, mt, :],
                                             rhs=h1[:, kt, :],
                                             start=(kt == 0),
                                             stop=(kt == HT - 1))
                        idx = e * HT + mt
                        nc.scalar.activation(h2[:, mt, :], ps[:], AF.Relu,
                                             bias=bnbsb[:, idx:idx + 1],
                                             scale=bnssb[:, idx:idx + 1])
                    eo = eopool.tile([128, CT, NSZ], F32)
                    for ct in range(CT):
                        ps = pspool.tile([128, NSZ], F32)
                        for kt in range(HT):
                            nc.tensor.matmul(ps[:],
                                             lhsT=w3sb[:, e, kt, ct, :],
                                             rhs=h2[:, kt, :],
                                             start=(kt == 0),
                                             stop=(kt == HT - 1))
                        idx = e * CT + ct
                        nc.scalar.add(eo[:, ct, :], ps[:],
                                      add=b3sb[:, idx:idx + 1])
                    pending = (eo, e, outg, j)
            peo, pe_, poutg, pj = pending
            emit_accums(peo, pe_, poutg)
            nc.sync.dma_start(out=outd[:, pj], in_=poutg[:])

    nc.compile()
    return nc


def _prep_inputs(x, gates, W1, b1, W2, b2, bn_gamma, bn_beta, bn_mean, bn_var,
                 W3, b3):
    """Host-side resharding of the full inputs into per-core in_maps."""
    f = np.float32
    # weights laid out exactly as the SBUF tiles expect:
    # w[p, e, kt, mt, m] = W[e, mt*128+m, kt*128+p]
    w1h = np.ascontiguousarray(
        np.asarray(W1, f).reshape(E, HT, 128, CT, 128).transpose(4, 0, 3, 1, 2))
    w2h = np.ascontiguousarray(
        np.asarray(W2, f).reshape(E, HT, 128, HT, 128).transpose(4, 0, 3, 1, 2))
    w3h = np.ascontiguousarray(
        np.asarray(W3, f).reshape(E, CT, 128, HT, 128).transpose(4, 0, 3, 1, 2))

    def col(v, nt):  # (E, nt*128) -> (128, E*nt):  out[p, e*nt+t] = v[e, t*128+p]
        return np.ascontiguousarray(
            np.asarray(v, f).reshape(E, nt, 128).transpose(2, 0, 1).reshape(128, E * nt))

    b1h = col(b1, HT)
    scale = np.asarray(bn_gamma, f) / np.sqrt(np.asarray(bn_var, f) + np.float32(BN_EPS))
    shift = (np.asarray(b2, f) - np.asarray(bn_mean, f)) * scale + np.asarray(bn_beta, f)
    bnsh = col(scale, HT)
    bnbh = col(shift, HT)
    b3h = col(b3, CT)
    # gt[p, kt, g*8+e] = gates[g, kt*128+p, e] / HW   (folds the GAP mean)
    gth = np.ascontiguousarray(
        (np.asarray(gates, f) / np.float32(HW))
        .transpose(1, 0, 2).reshape(CT, 128, G * E).transpose(1, 0, 2))

    # x[b] -> (128, NCH, CT, NSZ):  xh[p, j, kt, n] = x[b, kt*128+p, j*NSZ+n]
    xs = (np.asarray(x, f).reshape(B, CT, 128, NCH, NSZ)
          .transpose(0, 2, 3, 1, 4))
    common = dict(w1=w1h, w2=w2h, w3=w3h, b1s=b1h, bns=bnsh, bnb=bnbh,
                  b3s=b3h, gt=gth)
    return [dict(common, xb=np.ascontiguousarray(xs[b])) for b in range(B)]


def _assemble(results):
    outs = np.empty((G, B, C, H, W), np.float32)
    probs_all = np.empty((G, B, E), np.float32)
    for b in range(B):
        r = results[b]["out"]     # (128, NCH, G, CT, NSZ)
        outs[:, b] = (r.transpose(2, 3, 0, 1, 4)
                      .reshape(G, C, H, W))
        probs_all[:, b] = results[b]["probs"].reshape(G, E)
    usage = probs_all.mean(axis=0).mean(axis=0)
    loss = np.float32(np.var(usage, ddof=1) / (usage.mean() ** 2 + np.float32(1e-10)))
    return outs[0], outs[1], outs[2], outs[3], loss


def kernel(**inputs):
    if "nc" not in _CACHE:
        _CACHE["nc"] = _build_nc()
    nc = _CACHE["nc"]
    in_maps = _prep_inputs(**inputs)
    res = run_bass_kernel_spmd(nc, in_maps, core_ids=list(range(NCORES)))
    return _assemble(res.results)


# revision 15
# speedup vs baseline: 1.0450x; 1.0450x over previous
"""Trainium2 Bass kernel for the 4-gate top-2 MoE block (dense all-expert).

Strategy: pure data parallelism — sample b -> NeuronCore b (B == 8 == n_cores).
Each core runs the full gating + all-8-expert 1x1-conv stack + gated combine
for its own sample; outputs are concatenated on the host.  The load-balance
loss needs the softmax probs of all samples, so each core DMAs out its tiny
(4,8) probs tile and the final 8-value variance reduction happens on the host.

Matmuls run in float32r mode (4x the fp32 PE throughput).  The PE rounds
f32r operands internally; producers only have to be f32r-typed, so weights
and activations stream over the two hardware DGE rings as bitcast views —
no software (casting) DMA anywhere on the critical path.

DMA ring assignment:
  sync  (qSP HWDGE):  gates/biases, per-expert weights, per-chunk x (f32r)
  scalar(qAct HWDGE): per-chunk x (f32 copy for the exact global-avg-pool),
                      per-chunk output stores

Set MM_DTYPE = "f32" below for a full-precision fp32 fallback.
"""

import numpy as np

import concourse.bacc as bacc
import concourse.mybir as mybir
import concourse.tile as tile
from concourse.bass_utils import run_bass_kernel_spmd

F32 = mybir.dt.float32
F32R = mybir.dt.float32r
AF = mybir.ActivationFunctionType
ALU = mybir.AluOpType
AX = mybir.AxisListType

# problem shape (hardcoded; spec nn_MoE_77120432767217)
B, C, H, W = 8, 256, 56, 56
HW = H * W              # 3136
E, HID, G = 8, 512, 4
BN_EPS = 1e-5
CT = C // 128           # 2   c-tiles
HT = HID // 128         # 4   hid-tiles
NSZ = 392               # free-dim chunk
NCH = HW // NSZ         # 8   chunks
NCORES = 8

MM_DTYPE = "f32r"       # "f32r" (fast, tf32-ish) or "f32" (full precision)

_CACHE = {}


def _build_nc():
    """Build + compile the per-core Bass program (SPMD: same program, 8 cores)."""
    mmdt = F32R if MM_DTYPE == "f32r" else F32

    def asmm(ap):   # f32 dram view -> matmul-dtype view (free relabel)
        return ap.bitcast(mmdt) if mmdt != F32 else ap

    nc = bacc.Bacc("TRN2", target_bir_lowering=False, debug=False,
                   num_devices=NCORES)

    # ---- DRAM parameters (per core) ----
    xd = nc.dram_tensor("xb", [128, NCH, CT, NSZ], F32, kind="ExternalInput").ap()
    w1d = nc.dram_tensor("w1", [128, E, CT, HT, 128], F32, kind="ExternalInput").ap()
    w2d = nc.dram_tensor("w2", [128, E, HT, HT, 128], F32, kind="ExternalInput").ap()
    w3d = nc.dram_tensor("w3", [128, E, HT, CT, 128], F32, kind="ExternalInput").ap()
    b1d = nc.dram_tensor("b1s", [128, E * HT], F32, kind="ExternalInput").ap()
    bnsd = nc.dram_tensor("bns", [128, E * HT], F32, kind="ExternalInput").ap()
    bnbd = nc.dram_tensor("bnb", [128, E * HT], F32, kind="ExternalInput").ap()
    b3d = nc.dram_tensor("b3s", [128, E * CT], F32, kind="ExternalInput").ap()
    gtd = nc.dram_tensor("gt", [128, CT, G * E], F32, kind="ExternalInput").ap()
    outd = nc.dram_tensor("out", [128, NCH, G, CT, NSZ], F32,
                          kind="ExternalOutput").ap()
    probsd = nc.dram_tensor("probs", [1, G * E], F32, kind="ExternalOutput").ap()

    with tile.TileContext(nc) as tc:
        with (
            tc.tile_pool(name="wconst", bufs=1) as wpool,
            tc.tile_pool(name="bconst", bufs=1) as bpool,
            tc.tile_pool(name="xr", bufs=3) as xrpool,
            tc.tile_pool(name="xf", bufs=1) as xfpool,
            tc.tile_pool(name="gat", bufs=1) as gpool,
            tc.tile_pool(name="h1p", bufs=2) as h1pool,
            tc.tile_pool(name="h2p", bufs=2) as h2pool,
            tc.tile_pool(name="eop", bufs=2) as eopool,
            tc.tile_pool(name="outp", bufs=2) as outpool,
            tc.tile_pool(name="ps", bufs=7, space="PSUM") as pspool,
            tc.tile_pool(name="psg", bufs=1, space="PSUM") as psgpool,
        ):
            b1sb = bpool.tile([128, E * HT], F32, tag="b1sb")
            bnssb = bpool.tile([128, E * HT], F32, tag="bnssb")
            bnbsb = bpool.tile([128, E * HT], F32, tag="bnbsb")
            b3sb = bpool.tile([128, E * CT], F32, tag="b3sb")
            gtsb = bpool.tile([128, CT, G * E], F32, tag="gtsb")

            # ---- per-expert weights + rounded x chunks on the sync ring ----
            w1sb = wpool.tile([128, E, CT, HT, 128], mmdt, tag="w1sb")
            w2sb = wpool.tile([128, E, HT, HT, 128], mmdt, tag="w2sb")
            w3sb = wpool.tile([128, E, HT, CT, 128], mmdt, tag="w3sb")

            xrs = []

            # ---- x global-average-pool (exact fp32): two half-x DMAs, one
            # per HWDGE ring, reduced over (chunk, n) via transposed views --
            QJ = NCH // 4
            rsums = []

            def _xf_gap_stream():
                engs = [nc.sync, nc.scalar, nc.sync, nc.scalar]
                for h in range(4):
                    xf = xfpool.tile([128, QJ, CT, NSZ], F32, bufs=2)
                    engs[h].dma_start(
                        out=xf[:], in_=xd[:, h * QJ:(h + 1) * QJ])
                    rs = gpool.tile([128, CT], F32, tag=f"rs{h}")
                    nc.vector.tensor_reduce(out=rs[:],
                                            in_=xf[:].transpose([0, 2, 1, 3]),
                                            axis=AX.XY, op=ALU.add)
                    rsums.append(rs)

            def emit_xr(j):
                xr = xrpool.tile([128, CT, NSZ], mmdt, tag="xr")
                nc.sync.dma_start(out=xr[:], in_=asmm(xd[:, j]))
                xrs.append(xr)

            # warm the ACT table set containing Exp/Relu/Identity while
            # the first DMAs are in flight (saves ~2.7us on the gating chain)
            warm = gpool.tile([1, 1], F32, tag="warm")
            nc.vector.memset(warm[:], 0.0)
            nc.scalar.activation(warm[:], warm[:], AF.Exp)

            # ring heads: expert-0 weights + x chunk 0, then the GAP
            # quarters, gating constants, remaining weights.
            emit_xr(0)
            nc.sync.dma_start(out=w1sb[:, 0], in_=asmm(w1d[:, 0]))
            nc.sync.dma_start(out=w3sb[:, 0], in_=asmm(w3d[:, 0]))
            _xf_gap_stream()
            nc.sync.dma_start(out=gtsb[:], in_=gtd)
            # w2 rides the scalar ring in four 2-expert transfers: few ACT
            # trigger slots, but expert 0/1 weights still land early
            for p in range(4):
                nc.scalar.dma_start(out=w2sb[:, 2 * p:2 * p + 2],
                                    in_=asmm(w2d[:, 2 * p:2 * p + 2]))
            for t, d in ((b1sb, b1d), (bnssb, bnsd), (bnbsb, bnbd),
                         (b3sb, b3d)):
                nc.sync.dma_start(out=t[:], in_=d)
            for e in range(1, E):
                nc.sync.dma_start(out=w1sb[:, e], in_=asmm(w1d[:, e]))
                nc.sync.dma_start(out=w3sb[:, e], in_=asmm(w3d[:, e]))
                if e == 1:
                    emit_xr(1)
                if e == 4:
                    emit_xr(2)

            # ---- gating: logits = (sum_hw x / HW) @ gates ----
            xacc = gpool.tile([128, CT], F32, tag="xacc")
            nc.vector.tensor_add(out=xacc[:], in0=rsums[0][:], in1=rsums[1][:])
            for h in range(2, 4):
                nc.vector.tensor_add(out=xacc[:], in0=xacc[:], in1=rsums[h][:])

            lps = psgpool.tile([1, G * E], F32, tag="gps")
            for kt in range(CT):
                nc.tensor.matmul(lps[:], lhsT=xacc[:, kt:kt + 1],
                                 rhs=gtsb[:, kt, :],
                                 start=(kt == 0), stop=(kt == CT - 1))
            lg = gpool.tile([1, G * E], F32, tag="lg")
            nc.vector.tensor_copy(out=lg[:], in_=lps[:])

            def v48(t):   # (1, 32) tile -> (1, 4, 8) view
                return t[:].rearrange("p (g e) -> p g e", e=E)

            def bc(t):    # (1, 4) tile -> (1, 4, 8) broadcast view
                return t[:].unsqueeze(2).broadcast_to((1, G, E))

            # numerically-stable softmax over e (per gate)
            mx = gpool.tile([1, G], F32, tag="mx")
            nc.vector.tensor_reduce(out=mx[:], in_=v48(lg), axis=AX.X, op=ALU.max)
            sh = gpool.tile([1, G * E], F32, tag="sh")
            nc.vector.tensor_tensor(out=v48(sh), in0=v48(lg), in1=bc(mx),
                                    op=ALU.subtract)
            ex = gpool.tile([1, G * E], F32, tag="ex")
            nc.scalar.activation(ex[:], sh[:], AF.Exp)
            sm = gpool.tile([1, G], F32, tag="sm")
            nc.vector.tensor_reduce(out=sm[:], in_=v48(ex), axis=AX.X, op=ALU.add)
            sinv = gpool.tile([1, G], F32, tag="sinv")
            nc.vector.reciprocal(out=sinv[:], in_=sm[:])
            probs = gpool.tile([1, G * E], F32, tag="probs")
            nc.vector.tensor_tensor(out=v48(probs), in0=v48(ex), in1=bc(sinv),
                                    op=ALU.mult)
            nc.sync.dma_start(out=probsd, in_=probs[:])

            # top-2 per gate, selected on LOGITS (same order as probs,
            # but logit gaps are much wider than prob gaps -> robust)
            m1 = gpool.tile([1, G], F32, tag="m1")
            nc.vector.tensor_reduce(out=m1[:], in_=v48(lg), axis=AX.X, op=ALU.max)
            msk = gpool.tile([1, G * E], F32, tag="msk")
            nc.vector.tensor_tensor(out=v48(msk), in0=v48(lg), in1=bc(m1),
                                    op=ALU.is_equal)
            pm = gpool.tile([1, G * E], F32, tag="pm")
            nc.vector.scalar_tensor_tensor(out=v48(pm), in0=v48(msk),
                                           scalar=-1e30, in1=v48(lg),
                                           op0=ALU.mult, op1=ALU.add)
            m2 = gpool.tile([1, G], F32, tag="m2")
            nc.vector.tensor_reduce(out=m2[:], in_=v48(pm), axis=AX.X, op=ALU.max)
            sel = gpool.tile([1, G * E], F32, tag="sel")
            nc.vector.tensor_tensor(out=v48(sel), in0=v48(lg), in1=bc(m2),
                                    op=ALU.is_ge)
            num = gpool.tile([1, G * E], F32, tag="num")
            nc.vector.tensor_tensor(out=num[:], in0=probs[:], in1=sel[:],
                                    op=ALU.mult)
            den = gpool.tile([1, G], F32, tag="den")
            nc.vector.tensor_reduce(out=den[:], in_=v48(num), axis=AX.X, op=ALU.add)
            nc.vector.tensor_scalar_add(out=den[:], in0=den[:], scalar1=1e-10)
            dinv = gpool.tile([1, G], F32, tag="dinv")
            nc.vector.reciprocal(out=dinv[:], in_=den[:])
            cw = gpool.tile([1, G * E], F32, tag="cw")
            nc.vector.tensor_tensor(out=v48(cw), in0=v48(num), in1=bc(dinv),
                                    op=ALU.mult)

            ones = gpool.tile([1, 128], F32, tag="ones")
            nc.vector.memset(ones[:], 1.0)
            cwb = gpool.tile([128, G * E], F32, tag="cwb")

            def emit_cw_broadcast():
                # broadcast cw to all 128 partitions with a K=1 ones-matmul;
                # emitted a little way INTO the expert stream so the PE does
                # not sit on the softmax/top-k chain's latency.
                cwps = psgpool.tile([128, G * E], F32, tag="gps")
                nc.tensor.matmul(cwps[:], lhsT=ones[:], rhs=cw[:],
                                 start=True, stop=True)
                nc.vector.tensor_copy(out=cwb[:], in_=cwps[:])

            # scheduler fence: the gating phase schedules before the expert
            # loop on every engine, so the gated-combine's cw dependency can
            # never wedge the PSUM/PE pipeline below.
            tc.no_sync_barrier()

            # ---- main dense-expert stack, chunk-outer / expert-inner ----
            def emit_accums(eo, e, outg):
                # outg[:, g] (+)= cw[g,e] * eo (deferred by one expert)
                for g in range(G):
                    sc = cwb[:, g * E + e: g * E + e + 1]
                    eng = nc.vector
                    if e == 0:
                        eng.tensor_scalar_mul(out=outg[:, g], in0=eo[:],
                                              scalar1=sc)
                    else:
                        eng.scalar_tensor_tensor(out=outg[:, g], in0=eo[:],
                                                 scalar=sc, in1=outg[:, g],
                                                 op0=ALU.mult, op1=ALU.add)

            pending = None   # previous expert's eo, combined one expert late
            for j in range(NCH):
                if j + 3 < NCH:
                    emit_xr(j + 3)
                outg = outpool.tile([128, G, CT, NSZ], F32)
                for e in range(E):
                    h1 = h1pool.tile([128, HT, NSZ], mmdt)
                    for mt in range(HT):
                        ps = pspool.tile([128, NSZ], F32)
                        for kt in range(CT):
                            nc.tensor.matmul(ps[:],
                                             lhsT=w1sb[:, e, kt, mt, :],
                                             rhs=xrs[j][:, kt, :],
                                             start=(kt == 0),
                                             stop=(kt == CT - 1))
                        bia = b1sb[:, e * HT + mt: e * HT + mt + 1]
                        # e==0: keep DVE free for the previous chunk's
                        # deferred combine; otherwise alternate ACT/DVE
                        if e == 0 or mt % 2 == 0:
                            nc.scalar.add(h1[:, mt, :], ps[:], add=bia)
                        else:
                            nc.vector.tensor_scalar_add(out=h1[:, mt, :],
                                                        in0=ps[:], scalar1=bia)
                    if j == 0 and e == 1:
                        emit_cw_broadcast()
                    if pending is not None:
                        peo, pe_, poutg, pj = pending
                        emit_accums(peo, pe_, poutg)
                        if pe_ == E - 1:
                            nc.sync.dma_start(out=outd[:, pj], in_=poutg[:])
                        pending = None
                    h2 = h2pool.tile([128, HT, NSZ], mmdt)
                    for mt in range(HT):
                        ps = pspool.tile([128, NSZ], F32)
                        for kt in range(HT):
                            nc.tensor.matmul(ps[:],
                                             lhsT=w2sb[:, e, kt, mt, :],
                                             rhs=h1[:, kt, :],
                                             start=(kt == 0),
                                             stop=(kt == HT - 1))
                        idx = e * HT + mt
                        nc.scalar.activation(h2[:, mt, :], ps[:], AF.Relu,
                                             bias=bnbsb[:, idx:idx + 1],
                                             scale=bnssb[:, idx:idx + 1])
                    eo = eopool.tile([128, CT, NSZ], F32)
                    for ct in range(CT):
                        ps = pspool.tile([128, NSZ], F32)
                        for kt in range(HT):
                            nc.tensor.matmul(ps[:],
                                             lhsT=w3sb[:, e, kt, ct, :],
                                             rhs=h2[:, kt, :],
                                             start=(kt == 0),
                                             stop=(kt == HT - 1))
                        idx = e * CT + ct
                        nc.scalar.add(eo[:, ct, :], ps[:],
                                      add=b3sb[:, idx:idx + 1])
                    pending = (eo, e, outg, j)
            peo, pe_, poutg, pj = pending
            emit_accums(peo, pe_, poutg)
            nc.sync.dma_start(out=outd[:, pj], in_=poutg[:])

    nc.compile()
    return nc


def _prep_inputs(x, gates, W1, b1, W2, b2, bn_gamma, bn_beta, bn_mean, bn_var,
                 W3, b3):
    """Host-side resharding of the full inputs into per-core in_maps."""
    f = np.float32
    # weights laid out exactly as the SBUF tiles expect:
    # w[p, e, kt, mt, m] = W[e, mt*128+m, kt*128+p]
    w1h = np.ascontiguousarray(
        np.asarray(W1, f).reshape(E, HT, 128, CT, 128).transpose(4, 0, 3, 1, 2))
    w2h = np.ascontiguousarray(
        np.asarray(W2, f).reshape(E, HT, 128, HT, 128).transpose(4, 0, 3, 1, 2))
    w3h = np.ascontiguousarray(
        np.asarray(W3, f).reshape(E, CT, 128, HT, 128).transpose(4, 0, 3, 1, 2))

    def col(v, nt):  # (E, nt*128) -> (128, E*nt):  out[p, e*nt+t] = v[e, t*128+p]
        return np.ascontiguousarray(
            np.asarray(v, f).reshape(E, nt, 128).transpose(2, 0, 1).reshape(128, E * nt))

    b1h = col(b1, HT)
    scale = np.asarray(bn_gamma, f) / np.sqrt(np.asarray(bn_var, f) + np.float32(BN_EPS))
    shift = (np.asarray(b2, f) - np.asarray(bn_mean, f)) * scale + np.asarray(bn_beta, f)
    bnsh = col(scale, HT)
    bnbh = col(shift, HT)
    b3h = col(b3, CT)
    # gt[p, kt, g*8+e] = gates[g, kt*128+p, e] / HW   (folds the GAP mean)
    gth = np.ascontiguousarray(
        (np.asarray(gates, f) / np.float32(HW))
        .transpose(1, 0, 2).reshape(CT, 128, G * E).transpose(1, 0, 2))

    # x[b] -> (128, NCH, CT, NSZ):  xh[p, j, kt, n] = x[b, kt*128+p, j*NSZ+n]
    xs = (np.asarray(x, f).reshape(B, CT, 128, NCH, NSZ)
          .transpose(0, 2, 3, 1, 4))
    common = dict(w1=w1h, w2=w2h, w3=w3h, b1s=b1h, bns=bnsh, bnb=bnbh,
                  b3s=b3h, gt=gth)
    return [dict(common, xb=np.ascontiguousarray(xs[b])) for b in range(B)]


def _assemble(results):
    outs = np.empty((G, B, C, H, W), np.float32)
    probs_all = np.empty((G, B, E), np.float32)
    for b in range(B):
        r = results[b]["out"]     # (128, NCH, G, CT, NSZ)
        outs[:, b] = (r.transpose(2, 3, 0, 1, 4)
                      .reshape(G, C, H, W))
        probs_all[:, b] = results[b]["probs"].reshape(G, E)
    usage = probs_all.mean(axis=0).mean(axis=0)
    loss = np.float32(np.var(usage, ddof=1) / (usage.mean() ** 2 + np.float32(1e-10)))
    return outs[0], outs[1], outs[2], outs[3], loss


def kernel(**inputs):
    if "nc" not in _CACHE:
        _CACHE["nc"] = _build_nc()
    nc = _CACHE["nc"]
    in_maps = _prep_inputs(**inputs)
    res = run_bass_kernel_spmd(nc, in_maps, core_ids=list(range(NCORES)))
    return _assemble(res.results)
